# revision 30
# baseline (speedup 1.0000x reference)
"""Trainium2 Bass kernel for MTGNN temporal classifier (single layer).

Self-contained: takes FULL inputs as numpy arrays, shards across 8 NeuronCores
(batch x time-half), runs one SPMD Bass/Tile program, gathers the full output.

Sharding: core = 2*b + th  (b in 0..3 batches, th in 0..1 time-halves).
The mixprop hop GEMMs (dominant cost) run as FP8 DoubleRow matmuls (2x PE
throughput) in channel-major orientation: stationary = node-major h blocks,
moving = the fp8 normalized adjacency.  Only the attenuated A@h product sees
fp8 quantization noise; the dominant alpha*x re-add stays bf16.  Hops are
per-timestep independent, so the tau loop is software-pipelined (skew 3).
LayerNorm is folded analytically into the skipE convolution; the collectives
are pairwise ReduceScatters (each core finishes end-convs on its node half).
"""

import numpy as np
import ml_dtypes

import concourse.bass as bass
import concourse.tile as tile
import concourse.bass_isa as bass_isa
from concourse import bacc, mybir
from concourse import bass_utils

BF16 = mybir.dt.bfloat16
F32 = mybir.dt.float32
F8 = mybir.dt.float8e4
bf16 = ml_dtypes.bfloat16
f8e4 = ml_dtypes.float8_e4m3
AF = mybir.ActivationFunctionType
ALU = mybir.AluOpType
DR = mybir.MatmulPerfMode.DoubleRow

# problem dims
B, C_IN, N, T = 4, 129, 1259, 25
RC, CC, SC, EC, OUT = 128, 126, 128, 128, 64
K = 3
T1 = T - (K - 1)          # 23
NP = 1280                 # padded node count
NV = NP // 128            # 10 node blocks
NV2 = NV // 2             # 5 block pairs (fp8 DoubleRow)
TAU = 12                  # local output time steps per core (incl. 1 pad on th=1)
TLOC = TAU + 2            # 14 local input time steps
VCH = [(0, 512), (512, 512), (1024, 256)]   # v chunks (full NP)
NH = NP // 2              # node half per core after ReduceScatter
ECH = [(0, 512), (512, 128)]                # v chunks (node half)
CNT = float(RC * N * T1)  # layernorm element count per batch
EPS = 1e-5
SG = 2048.0               # fp8 scale on the normalized adjacency
SH = 16.0                 # fp8 scale on hop inputs h
INV = 1.0 / (SG * SH)

_CACHE = {}


def _build_program(debug_taps=False):
    nc = bacc.Bacc("TRN2", target_bir_lowering=False, debug=False, num_devices=8)

    def din(name, shape, dt=BF16):
        return nc.dram_tensor(name, shape, dt, kind="ExternalInput").ap()

    x_hi = din("x_hi", [128, TLOC, NP])
    x_lo = din("x_lo", [TLOC, NP])          # channel 128, [t, v]
    g8_1 = din("g8_1", [128, NV2, 2, NP], F8)   # SG * 0.5*norm_adj(adj).T packed
    g8_2 = din("g8_2", [128, NV2, 2, NP], F8)   # SG * 0.5*norm_adj(adj.T).T
    wsT_hi = din("wsT_hi", [128, 128])
    wsT_lo = din("wsT_lo", [1, 128])
    w0T_hi = din("w0T_hi", [128, TLOC, 128])
    w0T_lo = din("w0T_lo", [TLOC, 128])
    wfT = din("wfT", [128, K, 128])
    wf8 = din("wf8", [128, 2, 128], F8)
    wg8 = din("wg8", [128, 2, 128], F8)
    wgT = din("wgT", [128, K, 128])
    bf_v = din("bf_v", [128, 1], F32)
    bg_v = din("bg_v", [128, 1], F32)
    w1T = din("w1T", [CC, TAU, 128])        # host-scaled 2x (reads hx2 = x/2)
    wmp1T = din("wmp1T", [128, 4, 128])     # k=0 slot host-scaled 2x
    wmp2T = din("wmp2T", [128, 4, 128])
    b_resid_v = din("b_resid_v", [128, 1], F32)
    wET = din("wET", [128, TAU, 128])
    wEsum_v = din("wEsum_v", [128, 1], F32)
    b01_v = din("b01_v", [128, 1], F32)
    we1T = din("we1T", [128, 128])
    be1_v = din("be1_v", [128, 1], F32)
    we2T = din("we2T", [128, OUT])
    be2_v = din("be2_v", [OUT, 1], F32)
    whT = din("whT", [OUT, 1])
    bh_v = din("bh_v", [1, 1], F32)
    tmask = din("tmask", [128, TAU], F32)
    y = nc.dram_tensor("y", [1, NH], F32, kind="ExternalOutput").ap()
    taps = {}
    if debug_taps:
        for nm, shp, dt in [("d_hx2", [128, TAU, NP], BF16),
                            ("d_x2T8", [128, TAU, NV, 128], F8),
                            ("d_h1", [128, NP], BF16),
                            ("d_h2", [128, NP], BF16),
                            ("d_h3", [128, NP], BF16),
                            ("d_macc", [128, TAU, NP], BF16),
                            ("d_skip", [128, NP], BF16),
                            ("d_rawE", [128, NP], BF16),
                            ("d_stats", [128, 2], F32)]:
            taps[nm] = nc.dram_tensor(nm, shp, dt, kind="ExternalOutput").ap()

    with tile.TileContext(nc) as tc:
        with (
            tc.tile_pool(name="persist", bufs=1) as pp,
            tc.tile_pool(name="dram", bufs=1, space="DRAM") as dram,
        ):
            # ---- persistent tiles ----
            hx2 = pp.tile([128, TAU, NP], BF16)       # 0.5 * f * g, channel-major
            macc = pp.tile([128, TAU, NP], BF16)      # residual + mixprop accum
            skip_acc = pp.tile([128, NP], BF16)       # skip0+skip1 partial
            rawE_acc = pp.tile([128, NP], BF16)       # skipE on un-normalized h
            ones_c = pp.tile([128, 1], F32)           # partition-reduce helpers
            ones_r = pp.tile([1, 128], F32)
            x2T8 = pp.tile([128, TAU, NV, 128], F8)   # SH * x, node-major
            wmp1_t = pp.tile([128, 4, 128], BF16)
            wmp2_t = pp.tile([128, 4, 128], BF16)
            w1T_t = pp.tile([CC, TAU, 128], BF16)
            wET_t = pp.tile([128, TAU, 128], BF16)
            brv = pp.tile([128, 1], F32)
            wEs_t = pp.tile([128, 1], F32)
            b01_t = pp.tile([128, 1], F32)
            we1_t = pp.tile([128, 128], BF16)
            be1_t = pp.tile([128, 1], F32)
            we2_t = pp.tile([128, OUT], BF16)
            be2_t = pp.tile([OUT, 1], F32)
            whT_t = pp.tile([OUT, 1], BF16)
            bh_t = pp.tile([1, 1], F32)
            tmask_t = pp.tile([128, TAU], F32)
            sums_t = pp.tile([128, TAU], F32)
            sqs_t = pp.tile([128, TAU], F32)

            warm = pp.tile([1, 2], F32)
            warm_o = pp.tile([1, 2], F32)
            nc.vector.memset(warm[:], 0.0)

            nc.vector.memset(macc[:, :, N:NP], 0.0)
            nc.vector.memset(ones_c[:], 1.0)
            nc.vector.memset(ones_r[:], 1.0)

            ccw_in = dram.tile([2, 1, 2], F32)
            ccw_out = dram.tile([1, 2], F32)
            nc.gpsimd.dma_start(ccw_in[0, :, :], warm[:])
            nc.gpsimd.dma_start(ccw_in[1, :, :], warm[:])
            nc.gpsimd.collective_compute(
                "ReduceScatter", ALU.add,
                ins=[ccw_in.opt()], outs=[ccw_out.opt()],
                replica_groups=[[0, 1], [2, 3], [4, 5], [6, 7]])

            # ================= stage A =================
            with tc.tile_pool(name="stageA", bufs=1) as pa, \
                 tc.tile_pool(name="xhring", bufs=6) as pxh, \
                 tc.tile_pool(name="stag", bufs=4) as pstag, \
                 tc.tile_pool(name="fgring", bufs=3) as pfg, \
                 tc.tile_pool(name="tbring", bufs=2) as ptb:
                xlo_t = pa.tile([128, NP], BF16)
                H0 = pa.tile([128, TLOC, NP], BF16)
                ws_hi_t = pa.tile([128, 128], BF16)
                ws_lo_t = pa.tile([128, 128], BF16)
                w0_hi_t = pa.tile([128, TLOC, 128], BF16)
                w0_lo_t = pa.tile([128, 128], BF16)
                # rank-1 / t'-contraction operands are zero-padded to K=128 so
                # every matmul keeps one PE tile config: mixing (32,128) and
                # (128,128) tiles was measured to halve PE throughput
                nc.vector.memset(ws_lo_t[:], 0.0)
                nc.vector.memset(w0_lo_t[:], 0.0)
                nc.vector.memset(xlo_t[:], 0.0)
                wf_t = pa.tile([128, K, 128], BF16)
                wg_t = pa.tile([128, K, 128], BF16)
                wf8_t = pa.tile([128, 2, 128], F8)
                wg8_t = pa.tile([128, 2, 128], F8)
                H8 = pa.tile([128, TLOC, NP], F8)
                bfv_t = pa.tile([128, 1], F32)
                bgv_t = pa.tile([128, 1], F32)
                nc.sync.dma_start(wf8_t[:], wf8[:])
                nc.sync.dma_start(wg8_t[:], wg8[:])
                nc.sync.dma_start(xlo_t[0:TLOC, :], x_lo[:])
                nc.sync.dma_start(ws_lo_t[0:1, :], wsT_lo[:])
                nc.sync.dma_start(w0_lo_t[0:TLOC, :], w0T_lo[:])
                for t_, d_ in [(ws_hi_t, wsT_hi),
                               (w0_hi_t, w0T_hi),
                               (wf_t, wfT), (wg_t, wgT), (bfv_t, bf_v),
                               (bgv_t, bg_v)]:
                    nc.sync.dma_start(t_[:], d_[:])
                for t_, d_ in [(wmp1_t, wmp1T), (wmp2_t, wmp2T), (w1T_t, w1T),
                               (wET_t, wET), (brv, b_resid_v), (wEs_t, wEsum_v),
                               (b01_t, b01_v), (we1_t, we1T), (be1_t, be1_v),
                               (we2_t, we2T), (be2_t, be2_v), (whT_t, whT),
                               (bh_t, bh_v), (tmask_t, tmask)]:
                    nc.sync.dma_start(t_[:], d_[:])

                # phase 1: start conv (H0) + skip0, looped over t'
                with tc.tile_pool(name="psA1", bufs=5, space="PSUM") as psA1, \
                     tc.tile_pool(name="psA1s", bufs=1, space="PSUM") as psA1s:
                    s0ps = psA1s.tile([128, 3, 512], F32, tag="s0ps")
                    xh_tiles, stg_tiles = [], []
                    for tp_ in range(TLOC):
                        xh = pxh.tile([128, NP], BF16, tag="xh")
                        nc.scalar.dma_start(xh[:], x_hi[:, tp_, :])
                        xh_tiles.append(xh)
                        stg = pstag.tile([128, NP], BF16, tag="xlo_stage")
                        if tp_ < 4:
                            nc.gpsimd.memset(stg[:], 0.0)
                        nc.gpsimd.dma_start(stg[0:1, :], x_lo[tp_:tp_ + 1, :])
                        stg_tiles.append(stg)
                    for tp_ in range(TLOC):
                        xh, stg = xh_tiles[tp_], stg_tiles[tp_]
                        pss = []
                        for ci in range(3):
                            psum = psA1.tile([128, 512], F32, tag="ps_start",
                                             name="ps_start")
                            pss.append(psum)
                        # group matmuls by stationary to keep weight loads off
                        # the critical rate
                        for ci, (vo, vl) in enumerate(VCH):
                            nc.tensor.matmul(pss[ci][:, 0:vl], ws_hi_t[:],
                                             xh[:, vo:vo + vl],
                                             start=True, stop=False)
                        for ci, (vo, vl) in enumerate(VCH):
                            nc.tensor.matmul(pss[ci][:, 0:vl], ws_lo_t[:],
                                             stg[:, vo:vo + vl],
                                             start=False, stop=True)
                        for ci, (vo, vl) in enumerate(VCH):
                            nc.tensor.matmul(s0ps[:, ci, 0:vl], w0_hi_t[:, tp_, :],
                                             xh[:, vo:vo + vl],
                                             start=(tp_ == 0), stop=False)
                        for ci, (vo, vl) in enumerate(VCH):
                            if (tp_ * 3 + ci) % 2 == 0:
                                nc.scalar.activation(H0[:, tp_, vo:vo + vl],
                                                     pss[ci][:, 0:vl], AF.Copy)
                            else:
                                nc.vector.tensor_copy(H0[:, tp_, vo:vo + vl],
                                                      pss[ci][:, 0:vl])
                            nc.scalar.activation(H8[:, tp_, vo:vo + vl],
                                                 pss[ci][:, 0:vl], AF.Copy,
                                                 scale=16.0)
                    for ci, (vo, vl) in enumerate(VCH):
                        # channel-129 skip0 term: contraction over t' in one shot
                        nc.tensor.matmul(s0ps[:, ci, 0:vl], w0_lo_t[:],
                                         xlo_t[:, vo:vo + vl],
                                         start=False, stop=True)
                        nc.vector.tensor_copy(skip_acc[:, vo:vo + vl],
                                              s0ps[:, ci, 0:vl])

                # phase 2: filt/gate inception -> hx2 = 0.5*tanh(.)*sigmoid(.)
                with tc.tile_pool(name="psA2", bufs=6, space="PSUM") as psA2, \
                     tc.tile_pool(name="psA2s", bufs=2, space="PSUM") as psA2s:
                    for tau in range(TAU):
                        psfs, psgs, fss = [], [], []
                        for ci in range(3):
                            psf = psA2.tile([128, 512], F32, tag="ps_fg",
                                            name="ps_fg")
                            psfs.append(psf)
                        for ci, (vo, vl) in enumerate(VCH):
                            nc.tensor.matmul(psfs[ci][:, 0:vl], wf8_t[:],
                                             H8[:, tau:tau + 2, vo:vo + vl],
                                             start=True, stop=False, perf_mode=DR)
                        for ci, (vo, vl) in enumerate(VCH):
                            nc.tensor.matmul(psfs[ci][:, 0:vl], wf_t[:, 2, :],
                                             H0[:, tau + 2, vo:vo + vl],
                                             start=False, stop=True)
                        for ci, (vo, vl) in enumerate(VCH):
                            fs = pfg.tile([128, 512], BF16, tag="fs")
                            nc.scalar.activation(fs[:, 0:vl], psfs[ci][:, 0:vl],
                                                 AF.Tanh, bias=bfv_t[:],
                                                 scale=1.0 / 4096.0)
                            fss.append(fs)
                        for ci in range(3):
                            psg = psA2.tile([128, 512], F32, tag="ps_fg",
                                            name="ps_fg")
                            psgs.append(psg)
                        for ci, (vo, vl) in enumerate(VCH):
                            nc.tensor.matmul(psgs[ci][:, 0:vl], wg8_t[:],
                                             H8[:, tau:tau + 2, vo:vo + vl],
                                             start=True, stop=False, perf_mode=DR)
                        for ci, (vo, vl) in enumerate(VCH):
                            nc.tensor.matmul(psgs[ci][:, 0:vl], wg_t[:, 2, :],
                                             H0[:, tau + 2, vo:vo + vl],
                                             start=False, stop=True)
                        for ci, (vo, vl) in enumerate(VCH):
                            gs = pfg.tile([128, 512], BF16, tag="gs")
                            nc.scalar.activation(gs[:, 0:vl], psgs[ci][:, 0:vl],
                                                 AF.Sigmoid, bias=bgv_t[:],
                                                 scale=1.0 / 4096.0)
                            nc.vector.scalar_tensor_tensor(
                                hx2[:, tau, vo:vo + vl], fss[ci][:, 0:vl], 0.5,
                                gs[:, 0:vl], ALU.mult, ALU.mult)
                        # node-major fp8 copy of x (hop-1 stationary), per tau
                        tb = ptb.tile([128, NV, 128], BF16, tag="xtb")
                        nc.sync.dma_start_transpose(tb[:], hx2[:, tau, :])
                        nc.vector.tensor_scalar_mul(x2T8[:, tau, :, :], tb[:],
                                                    2.0 * SH)

                    # residual (+ start/mixprop biases) into macc, real nodes
                    nc.vector.tensor_scalar_add(macc[:, :, 0:N],
                                                H0[:, 2:TLOC, 0:N], brv[:])
                    if debug_taps:
                        nc.gpsimd.dma_start(taps["d_hx2"][:], hx2[:])
                        nc.gpsimd.dma_start(taps["d_x2T8"][:], x2T8[:])

                    # skip1 conv partial (contract c,tau over local range)
                    for vo, vl in VCH:
                        psum = psA2s.tile([128, 512], F32, tag="ps_s1")
                        for tau in range(TAU):
                            nc.tensor.matmul(psum[:, 0:vl], w1T_t[:, tau, :],
                                             hx2[0:CC, tau, vo:vo + vl],
                                             start=(tau == 0), stop=(tau == TAU - 1))
                        nc.vector.tensor_tensor(skip_acc[:, vo:vo + vl],
                                                skip_acc[:, vo:vo + vl],
                                                psum[:, 0:vl], op=ALU.add)

            # stage skip01 partials into the (single, tail) collective payload:
            # cols [0:NH]=rawE half, [NH:2NH]=skip half, [2NH:2NH+2]=stats
            cc_in = dram.tile([2, 128, 2 * NH + 2], BF16)
            cc_out = dram.tile([128, 2 * NH + 2], BF16)
            nc.gpsimd.dma_start(cc_in[0, :, NH:2 * NH], skip_acc[:, 0:NH])
            nc.gpsimd.dma_start(cc_in[1, :, NH:2 * NH], skip_acc[:, NH:NP])

            # ================= mixprop =================
            # per-tau chains h_{k+1} = G h_k + 0.5x, software-pipelined (skew 3):
            # step t issues hop1(t), hop2(t-1), hop3(t-2), conv1x1+stats(t-3)
            with tc.tile_pool(name="mx", bufs=4) as mx, \
                 tc.tile_pool(name="mxg", bufs=1) as mxg, \
                 tc.tile_pool(name="mxb", bufs=3) as mxb, \
                 tc.tile_pool(name="mx8", bufs=4) as mx8, \
                 tc.tile_pool(name="mscr", bufs=2) as mscr, \
                 tc.tile_pool(name="psH", bufs=4, space="PSUM") as psH, \
                 tc.tile_pool(name="psC", bufs=1, space="PSUM") as psC, \
                 tc.tile_pool(name="psR", bufs=1, space="PSUM") as psR:

                g8a_t = mxg.tile([128, NV2, 2, NP], F8, tag="g8a", name="g8a_t")
                g8b_t = mxg.tile([128, NV2, 2, NP], F8, tag="g8b", name="g8b_t")
                nc.gpsimd.dma_start(g8a_t[:], g8_1[:])
                nc.gpsimd.dma_start(g8b_t[:], g8_2[:])

                def emit_hop(g8, stat, hc, kk, tau, mp):
                    # hc[:, kk, :] = INV * (stat.T-pairs @ g8) + hx2[:, tau, :]
                    for vo, vl in VCH:
                        ps = psH.tile([128, 512], F32, tag="ps_hop")
                        for m in range(NV2):
                            nc.tensor.matmul(ps[:, 0:vl],
                                             stat[:, 2 * m:2 * m + 2, :],
                                             g8[:, m, :, vo:vo + vl],
                                             start=(m == 0), stop=(m == NV2 - 1),
                                             perf_mode=DR)
                        nc.vector.scalar_tensor_tensor(
                            hc[:, kk, vo:vo + vl], ps[:, 0:vl], INV,
                            hx2[:, tau, vo:vo + vl], ALU.mult, ALU.add)
                    if debug_taps and mp == 0 and tau == 0:
                        nc.gpsimd.dma_start(taps[f"d_h{kk + 1}"][:], hc[:, kk, :])
                    if kk == 2:
                        return None
                    # node-major fp8 copy for the next hop's stationary
                    tb = mxb.tile([128, NV, 128], BF16, tag="htb")
                    nc.sync.dma_start_transpose(tb[:], hc[:, kk, :])
                    t8 = mx8.tile([128, NV, 128], F8, tag="ht8")
                    nc.scalar.activation(t8[:], tb[:], AF.Copy, scale=SH)
                    return t8

                for mp in range(2):
                    g8 = g8a_t if mp == 0 else g8b_t
                    wmp = wmp1_t if mp == 0 else wmp2_t
                    st = {}
                    rEps = {}
                    for step in range(TAU + 3):
                        t1 = step
                        if t1 < TAU:
                            hc = mx.tile([128, 3, NP], BF16, tag="hcm3")
                            st[t1] = [hc, None, None]
                            st[t1][1] = emit_hop(g8, x2T8[:, t1, :, :], hc, 0,
                                                 t1, mp)
                        t2 = step - 1
                        if 0 <= t2 < TAU:
                            hc = st[t2][0]
                            st[t2][2] = emit_hop(g8, st[t2][1][:], hc, 1, t2, mp)
                        t3 = step - 2
                        if 0 <= t3 < TAU:
                            emit_hop(g8, st[t3][2][:], st[t3][0], 2, t3, mp)
                        t4 = step - 3
                        if 0 <= t4 < TAU:
                            hc = st.pop(t4)[0]
                            for vo, vl in VCH:
                                ps = psC.tile([128, 512], F32, tag="ps_cv")
                                if mp == 0:
                                    nc.tensor.matmul(ps[:, 0:vl], wmp[:, 0, :],
                                                     hx2[:, t4, vo:vo + vl],
                                                     start=True, stop=False)
                                for k in range(3):
                                    nc.tensor.matmul(ps[:, 0:vl], wmp[:, k + 1, :],
                                                     hc[:, k, vo:vo + vl],
                                                     start=(mp == 1 and k == 0),
                                                     stop=(k == 2))
                                hi = min(vo + vl, N)
                                nc.vector.tensor_tensor(
                                    macc[:, t4, vo:hi], macc[:, t4, vo:hi],
                                    ps[:, 0:hi - vo], op=ALU.add)
                            if mp == 1:
                                # macc row t4 final: stats + rawE contribution
                                nc.vector.reduce_sum(sums_t[:, t4:t4 + 1],
                                                     macc[:, t4, :],
                                                     axis=mybir.AxisListType.X)
                                scr = mscr.tile([128, NP], BF16, tag="sq_scr")
                                nc.scalar.activation(scr[:], macc[:, t4, :],
                                                     AF.Square,
                                                     scale=tmask_t[:, t4:t4 + 1],
                                                     accum_out=sqs_t[:, t4:t4 + 1])
                                for ci, (vo, vl) in enumerate(VCH):
                                    if t4 == 0:
                                        rEps[ci] = psR.tile(
                                            [128, 512], F32,
                                            tag=f"ps_rE{ci}", name=f"ps_rE{ci}")
                                    nc.tensor.matmul(rEps[ci][:, 0:vl],
                                                     wET_t[:, t4, :],
                                                     macc[:, t4, vo:vo + vl],
                                                     start=(t4 == 0),
                                                     stop=(t4 == TAU - 1))
                                    if t4 == TAU - 1:
                                        nc.vector.tensor_copy(
                                            rawE_acc[:, vo:vo + vl],
                                            rEps[ci][:, 0:vl])

            if debug_taps:
                nc.gpsimd.dma_start(taps["d_macc"][:], macc[:])
                nc.gpsimd.dma_start(taps["d_skip"][:], skip_acc[:])
                nc.gpsimd.dma_start(taps["d_rawE"][:], rawE_acc[:])
            # re-warm act tables for the tail under the collective's shadow
            nc.scalar.activation(warm_o[:], warm[:], AF.Sqrt)
            # ================= stats + collective + end convs =================
            with tc.tile_pool(name="late", bufs=1) as pl, \
                 tc.tile_pool(name="psL", bufs=1, space="PSUM") as ps:
                stats_p = pl.tile([128, 2], F32)
                msum = pl.tile([128, TAU], F32)
                nc.vector.tensor_tensor(msum[:], sums_t[:], tmask_t[:], op=ALU.mult)
                nc.vector.reduce_sum(stats_p[:, 0:1], msum[:],
                                     axis=mybir.AxisListType.X)
                nc.vector.reduce_sum(stats_p[:, 1:2], sqs_t[:],
                                     axis=mybir.AxisListType.X)
                if debug_taps:
                    nc.gpsimd.dma_start(taps["d_stats"][:], stats_p[:])

                # cross-partition reduce + broadcast of stats on the PE
                # (avoids a gpsimd custom-op library reload on the tail)
                ps_s0 = ps.tile([1, 2], F32, tag="ps_s0")
                nc.tensor.matmul(ps_s0[:], ones_c[:], stats_p[:],
                                 start=True, stop=True)
                st01 = pl.tile([1, 2], F32)
                nc.vector.tensor_copy(st01[:], ps_s0[:])
                ps_s1 = ps.tile([128, 2], F32, tag="ps_s1")
                nc.tensor.matmul(ps_s1[:], ones_r[:], st01[:],
                                 start=True, stop=True)
                st_loc = pl.tile([128, 2], BF16)
                nc.vector.tensor_copy(st_loc[:], ps_s1[:])

                nc.gpsimd.dma_start(cc_in[0, :, 0:NH], rawE_acc[:, 0:NH])
                nc.gpsimd.dma_start(cc_in[1, :, 0:NH], rawE_acc[:, NH:NP])
                nc.gpsimd.dma_start(cc_in[0, :, 2 * NH:2 * NH + 2], st_loc[:])
                nc.gpsimd.dma_start(cc_in[1, :, 2 * NH:2 * NH + 2], st_loc[:])
                nc.gpsimd.collective_compute(
                    "ReduceScatter", ALU.add,
                    ins=[cc_in.opt()], outs=[cc_out.opt()],
                    replica_groups=[[0, 1], [2, 3], [4, 5], [6, 7]])
                rawE_c = pl.tile([128, 2 * NH + 2], BF16)
                nc.gpsimd.dma_start(rawE_c[:, 2 * NH:2 * NH + 2],
                                    cc_out[:, 2 * NH:2 * NH + 2])
                nc.scalar.dma_start(rawE_c[:, 0:2 * NH], cc_out[:, 0:2 * NH])

                # layernorm scalars (same value on every partition)
                mv = pl.tile([128, 1], F32)
                msqv = pl.tile([128, 1], F32)
                varv = pl.tile([128, 1], F32)
                m2v = pl.tile([128, 1], F32)
                svv = pl.tile([128, 1], F32)
                rv = pl.tile([128, 1], F32)
                rmv = pl.tile([128, 1], F32)
                bias_c = pl.tile([128, 1], F32)
                nc.vector.tensor_scalar_mul(mv[:], rawE_c[:, 2 * NH:2 * NH + 1],
                                            1.0 / CNT)
                nc.vector.tensor_scalar_mul(msqv[:],
                                            rawE_c[:, 2 * NH + 1:2 * NH + 2],
                                            1.0 / CNT)
                nc.vector.tensor_tensor(m2v[:], mv[:], mv[:], op=ALU.mult)
                nc.vector.tensor_scalar(varv[:], msqv[:], m2v[:], EPS,
                                        op0=ALU.subtract, op1=ALU.add)
                nc.scalar.sqrt(svv[:], varv[:])
                nc.vector.reciprocal(rv[:], svv[:])
                nc.vector.tensor_scalar(rmv[:], rv[:], mv[:], -1.0,
                                        op0=ALU.mult, op1=ALU.mult)
                # bias_c = b01 - r*m*wEsum
                nc.vector.scalar_tensor_tensor(bias_c[:], wEs_t[:], rmv[:],
                                               b01_t[:], ALU.mult, ALU.add)
                # skip_pre = skip01 + r*rawE ; relu with bias; end convs +
                # head, pipelined per node chunk so the serial chain overlaps
                skip_pre = pl.tile([128, NH], F32)
                rsk = pl.tile([128, NH], BF16)
                o1 = pl.tile([128, NH], BF16)
                o2 = pl.tile([OUT, NH], BF16)
                y_sb = pl.tile([1, NH], F32)
                ps1 = ps.tile([128, NH], F32, tag="ps_e1")
                ps2 = ps.tile([OUT, NH], F32, tag="ps_e2")
                psh = ps.tile([1, NH], F32, tag="ps_eh")
                for vo, vl in ECH:
                    nc.vector.scalar_tensor_tensor(
                        skip_pre[:, vo:vo + vl], rawE_c[:, vo:vo + vl],
                        rv[:], rawE_c[:, NH + vo:NH + vo + vl],
                        ALU.mult, ALU.add)
                    nc.vector.tensor_scalar(rsk[:, vo:vo + vl],
                                            skip_pre[:, vo:vo + vl],
                                            bias_c[:], 0.0,
                                            op0=ALU.add, op1=ALU.max)
                for vo, vl in ECH:
                    nc.tensor.matmul(ps1[:, vo:vo + vl], we1_t[:],
                                     rsk[:, vo:vo + vl], start=True, stop=True)
                    nc.vector.tensor_scalar(o1[:, vo:vo + vl],
                                            ps1[:, vo:vo + vl], be1_t[:], 0.0,
                                            op0=ALU.add, op1=ALU.max)
                for vo, vl in ECH:
                    nc.tensor.matmul(ps2[:, vo:vo + vl], we2_t[:],
                                     o1[:, vo:vo + vl], start=True, stop=True)
                    nc.vector.tensor_scalar_add(o2[:, vo:vo + vl],
                                                ps2[:, vo:vo + vl], be2_t[:])
                for vo, vl in ECH:
                    nc.tensor.matmul(psh[:, vo:vo + vl], whT_t[:],
                                     o2[:, vo:vo + vl], start=True, stop=True)
                nc.scalar.activation(y_sb[:], psh[:], AF.Sigmoid,
                                     bias=bh_t[:], scale=1.0)
                nc.gpsimd.dma_start(y[:], y_sb[:])

    nc.compile()
    return nc


def _wk2(w):
    # k-taps as [c, k, o(pad 128)]; tap 2 carries the 4096x psum scale used by
    # the fp8 DoubleRow path for taps 0/1
    arr = np.pad(w.transpose(1, 2, 0), ((0, 0), (0, 0), (0, 2)))
    arr[:, 2, :] *= 4096.0
    return arr.astype(bf16)


def _w8(w):
    arr = np.pad(w.transpose(1, 2, 0)[:, 0:2, :], ((0, 0), (0, 0), (0, 2)))
    return np.clip(arr * 256.0, -240.0, 240.0).astype(f8e4)


def _norm_adj_T_g8(a):
    """SG * 0.5 * norm_adj(a).T zero-padded, packed for fp8 DoubleRow rhs."""
    an = a + np.eye(N, dtype=np.float32)
    an = an / an.sum(axis=1, keepdims=True)
    g = (0.5 * SG) * an.T
    gp = np.zeros((NP, NP), dtype=np.float32)
    gp[:N, :N] = g
    np.clip(gp, -240.0, 240.0, out=gp)
    # w = 256*m + 128*j + p  ->  [p, m, j, v]
    return gp.reshape(NV2, 2, 128, NP).transpose(2, 0, 1, 3).astype(f8e4)


def _prep_inputs(inputs):
    x = np.asarray(inputs["x"], np.float32)
    adj = np.asarray(inputs["adj"], np.float32)
    w_start = np.asarray(inputs["w_start"], np.float32)
    b_start = np.asarray(inputs["b_start"], np.float32)
    w_filt = np.asarray(inputs["w_filt"], np.float32)[:, :, 0, :]
    b_filt = np.asarray(inputs["b_filt"], np.float32)
    w_gate = np.asarray(inputs["w_gate"], np.float32)[:, :, 0, :]
    b_gate = np.asarray(inputs["b_gate"], np.float32)
    w_skip0 = np.asarray(inputs["w_skip0"], np.float32)[:, :, 0, :]
    b_skip0 = np.asarray(inputs["b_skip0"], np.float32)
    w_skip1 = np.asarray(inputs["w_skip1"], np.float32)[:, :, 0, :]
    b_skip1 = np.asarray(inputs["b_skip1"], np.float32)
    w_mp1 = np.asarray(inputs["w_mp1"], np.float32)
    b_mp1 = np.asarray(inputs["b_mp1"], np.float32)
    w_mp2 = np.asarray(inputs["w_mp2"], np.float32)
    b_mp2 = np.asarray(inputs["b_mp2"], np.float32)
    w_skipE = np.asarray(inputs["w_skipE"], np.float32)[:, :, 0, :]
    b_skipE = np.asarray(inputs["b_skipE"], np.float32)
    w_end1 = np.asarray(inputs["w_end1"], np.float32)
    b_end1 = np.asarray(inputs["b_end1"], np.float32)
    w_end2 = np.asarray(inputs["w_end2"], np.float32)
    b_end2 = np.asarray(inputs["b_end2"], np.float32)
    w_head = np.asarray(inputs["w_head"], np.float32)
    b_head = np.asarray(inputs["b_head"], np.float32)

    g8_1 = _norm_adj_T_g8(adj)
    g8_2 = _norm_adj_T_g8(adj.T)

    # shared (core-independent) tensors
    wsT = w_start.T  # [129, 128]
    shared = {
        "g8_1": g8_1, "g8_2": g8_2,
        "wsT_hi": wsT[:128].astype(bf16),
        "wsT_lo": wsT[128:129].astype(bf16),
        "wfT": _wk2(w_filt), "wgT": _wk2(w_gate),
        "wf8": _w8(w_filt), "wg8": _w8(w_gate),
        "bf_v": np.pad((b_filt + w_filt.sum(2) @ b_start), (0, 2)).reshape(128, 1).astype(np.float32),
        "bg_v": np.pad((b_gate + w_gate.sum(2) @ b_start), (0, 2)).reshape(128, 1).astype(np.float32),
        "b_resid_v": (b_start + b_mp1 + b_mp2).reshape(128, 1).astype(np.float32),
        "wEsum_v": w_skipE.sum((1, 2)).reshape(128, 1).astype(np.float32),
        "b01_v": (b_skip0 + b_skip1 + b_skipE).reshape(128, 1).astype(np.float32),
        "we1T": w_end1.T.astype(bf16),
        "be1_v": b_end1.reshape(128, 1).astype(np.float32),
        "we2T": w_end2.T.astype(bf16),
        "be2_v": b_end2.reshape(OUT, 1).astype(np.float32),
        "whT": w_head.T.astype(bf16),
        "bh_v": b_head.reshape(1, 1).astype(np.float32),
    }
    # w_mp as [c(128 pad), k, o]; k=0 slot 2x (it multiplies hx2 = x/2) and
    # carries BOTH mixprops' x-terms (applied once, in the mp=0 conv)
    for nm, w, wo in (("wmp1T", w_mp1, w_mp2), ("wmp2T", w_mp2, None)):
        arr = np.zeros((128, 4, 128), np.float32)
        for k in range(4):
            arr[:CC, k, :] = w[:, k * CC:(k + 1) * CC].T
        arr[:, 0, :] *= 2.0
        if wo is not None:
            arr[:CC, 0, :] += 2.0 * wo[:, 0:CC].T
        shared[nm] = arr.astype(bf16)

    in_maps = []
    for core in range(8):
        b, th = core // 2, core % 2
        t_lo = 0 if th == 0 else TAU
        # x slice [129, 1280, TLOC] zero-padded in nodes and t
        xp = np.zeros((C_IN, TLOC, NP), np.float32)
        t_hi = min(t_lo + TLOC, T)
        xp[:, 0:t_hi - t_lo, :N] = x[b, :, :, t_lo:t_hi].transpose(0, 2, 1)
        # skip0 weight slots aligned to local t: core owns t range
        w0T = np.zeros((C_IN, TLOC, 128), np.float32)
        own_lo, own_hi = (0, 13) if th == 0 else (13, T)
        for tp_ in range(TLOC):
            tg = t_lo + tp_
            if own_lo <= tg < own_hi:
                w0T[:, tp_, :] = w_skip0[:, :, tg].T
        # skip1 / skipE weight slots aligned to local tau
        w1Ta = np.zeros((CC, TAU, 128), np.float32)
        wETa = np.zeros((128, TAU, 128), np.float32)
        for tau in range(TAU):
            tg = t_lo + tau
            if tg < T1:
                w1Ta[:, tau, :] = 2.0 * w_skip1[:, :, tg].T  # reads hx2 = x/2
                wETa[:, tau, :] = w_skipE[:, :, tg].T
        tm = np.ones((128, TAU), np.float32)
        if th == 1:
            tm[:, T1 - TAU:] = 0.0  # tau slots beyond T1 are padding
        m = dict(shared)
        m["x_hi"] = xp[:128].astype(bf16)
        m["x_lo"] = xp[128].astype(bf16)
        m["w0T_hi"] = w0T[:128].astype(bf16)
        m["w0T_lo"] = w0T[128].astype(bf16)
        m["w1T"] = w1Ta.astype(bf16)
        m["wET"] = wETa.astype(bf16)
        m["tmask"] = tm
        in_maps.append(m)
    return in_maps


def kernel(**inputs):
    if "nc" not in _CACHE:
        _CACHE["nc"] = _build_program()
    nc = _CACHE["nc"]
    in_maps = _prep_inputs(inputs)
    res = bass_utils.run_bass_kernel_spmd(nc, in_maps, core_ids=list(range(8)))
    out = np.empty((B, N), np.float32)
    for b in range(B):
        out[b, 0:NH] = res.results[2 * b]["y"][0]
        out[b, NH:N] = res.results[2 * b + 1]["y"][0, 0:N - NH]
    return out


# revision 31
# speedup vs baseline: 1.0119x; 1.0119x over previous
"""Trainium2 Bass kernel for MTGNN temporal classifier (single layer).

Self-contained: takes FULL inputs as numpy arrays, shards across 8 NeuronCores
(batch x time-half), runs one SPMD Bass/Tile program, gathers the full output.

Sharding: core = 2*b + th  (b in 0..3 batches, th in 0..1 time-halves).
The mixprop hop GEMMs (dominant cost) run as FP8 DoubleRow matmuls (2x PE
throughput) in channel-major orientation: stationary = node-major h blocks,
moving = the fp8 normalized adjacency.  Only the attenuated A@h product sees
fp8 quantization noise; the dominant alpha*x re-add stays bf16.  Hops are
per-timestep independent, so the tau loop is software-pipelined (skew 3).
LayerNorm is folded analytically into the skipE convolution; the collectives
are pairwise ReduceScatters (each core finishes end-convs on its node half).
"""

import numpy as np
import ml_dtypes

import concourse.bass as bass
import concourse.tile as tile
import concourse.bass_isa as bass_isa
from concourse import bacc, mybir
from concourse import bass_utils

BF16 = mybir.dt.bfloat16
F32 = mybir.dt.float32
F8 = mybir.dt.float8e4
bf16 = ml_dtypes.bfloat16
f8e4 = ml_dtypes.float8_e4m3
AF = mybir.ActivationFunctionType
ALU = mybir.AluOpType
DR = mybir.MatmulPerfMode.DoubleRow

# problem dims
B, C_IN, N, T = 4, 129, 1259, 25
RC, CC, SC, EC, OUT = 128, 126, 128, 128, 64
K = 3
T1 = T - (K - 1)          # 23
NP = 1280                 # padded node count
NV = NP // 128            # 10 node blocks
NV2 = NV // 2             # 5 block pairs (fp8 DoubleRow)
TAU = 12                  # local output time steps per core (incl. 1 pad on th=1)
TLOC = TAU + 2            # 14 local input time steps
VCH = [(0, 512), (512, 512), (1024, 256)]   # v chunks (full NP)
NH = NP // 2              # node half per core after ReduceScatter
ECH = [(0, 512), (512, 128)]                # v chunks (node half)
CNT = float(RC * N * T1)  # layernorm element count per batch
EPS = 1e-5
SG = 2048.0               # fp8 scale on the normalized adjacency
SH = 16.0                 # fp8 scale on hop inputs h
INV = 1.0 / (SG * SH)

_CACHE = {}


def _build_program(debug_taps=False):
    nc = bacc.Bacc("TRN2", target_bir_lowering=False, debug=False, num_devices=8)

    def din(name, shape, dt=BF16):
        return nc.dram_tensor(name, shape, dt, kind="ExternalInput").ap()

    x_hi = din("x_hi", [128, TLOC, NP])
    x_lo = din("x_lo", [TLOC, NP])          # channel 128, [t, v]
    g8_1 = din("g8_1", [128, NV2, 2, NP], F8)   # SG * 0.5*norm_adj(adj).T packed
    g8_2 = din("g8_2", [128, NV2, 2, NP], F8)   # SG * 0.5*norm_adj(adj.T).T
    wsT_hi = din("wsT_hi", [128, 128])
    wsT_lo = din("wsT_lo", [1, 128])
    w0T_hi = din("w0T_hi", [128, TLOC, 128])
    w0T_lo = din("w0T_lo", [TLOC, 128])
    wfT = din("wfT", [128, K, 128])
    wf8 = din("wf8", [128, 2, 128], F8)
    wg8 = din("wg8", [128, 2, 128], F8)
    wgT = din("wgT", [128, K, 128])
    bf_v = din("bf_v", [128, 1], F32)
    bg_v = din("bg_v", [128, 1], F32)
    w1T = din("w1T", [CC, TAU, 128])        # host-scaled 2x (reads hx2 = x/2)
    wmp1T = din("wmp1T", [128, 4, 128])     # k=0 slot host-scaled 2x
    wmp2T = din("wmp2T", [128, 4, 128])
    b_resid_v = din("b_resid_v", [128, 1], F32)
    wET = din("wET", [128, TAU, 128])
    wEsum_v = din("wEsum_v", [128, 1], F32)
    b01_v = din("b01_v", [128, 1], F32)
    we1T = din("we1T", [128, 128])
    be1_v = din("be1_v", [128, 1], F32)
    we2T = din("we2T", [128, OUT])
    be2_v = din("be2_v", [OUT, 1], F32)
    whT = din("whT", [OUT, 1])
    bh_v = din("bh_v", [1, 1], F32)
    tmask = din("tmask", [128, TAU], F32)
    y = nc.dram_tensor("y", [1, NH], F32, kind="ExternalOutput").ap()
    taps = {}
    if debug_taps:
        for nm, shp, dt in [("d_hx2", [128, TAU, NP], BF16),
                            ("d_x2T8", [128, TAU, NV, 128], F8),
                            ("d_h1", [128, NP], BF16),
                            ("d_h2", [128, NP], BF16),
                            ("d_h3", [128, NP], BF16),
                            ("d_macc", [128, TAU, NP], BF16),
                            ("d_skip", [128, NP], BF16),
                            ("d_rawE", [128, NP], BF16),
                            ("d_stats", [128, 2], F32)]:
            taps[nm] = nc.dram_tensor(nm, shp, dt, kind="ExternalOutput").ap()

    with tile.TileContext(nc) as tc:
        with (
            tc.tile_pool(name="persist", bufs=1) as pp,
            tc.tile_pool(name="dram", bufs=1, space="DRAM") as dram,
        ):
            # ---- persistent tiles ----
            hx2 = pp.tile([128, TAU, NP], BF16)       # 0.5 * f * g, channel-major
            macc = pp.tile([128, TAU, NP], BF16)      # residual + mixprop accum
            skip_acc = pp.tile([128, NP], BF16)       # skip0+skip1 partial
            rawE_acc = pp.tile([128, NP], BF16)       # skipE on un-normalized h
            ones_c = pp.tile([128, 1], F32)           # partition-reduce helpers
            ones_r = pp.tile([1, 128], F32)
            x2T8 = pp.tile([128, TAU, NV, 128], F8)   # SH * x, node-major
            wmp1_t = pp.tile([128, 4, 128], BF16)
            wmp2_t = pp.tile([128, 4, 128], BF16)
            w1T_t = pp.tile([CC, TAU, 128], BF16)
            wET_t = pp.tile([128, TAU, 128], BF16)
            brv = pp.tile([128, 1], F32)
            wEs_t = pp.tile([128, 1], F32)
            b01_t = pp.tile([128, 1], F32)
            we1_t = pp.tile([128, 128], BF16)
            be1_t = pp.tile([128, 1], F32)
            we2_t = pp.tile([128, OUT], BF16)
            be2_t = pp.tile([OUT, 1], F32)
            whT_t = pp.tile([OUT, 1], BF16)
            bh_t = pp.tile([1, 1], F32)
            tmask_t = pp.tile([128, TAU], F32)
            sums_t = pp.tile([128, TAU], F32)
            sqs_t = pp.tile([128, TAU], F32)

            warm = pp.tile([1, 2], F32)
            warm_o = pp.tile([1, 2], F32)
            nc.vector.memset(warm[:], 0.0)

            nc.vector.memset(macc[:, :, N:NP], 0.0)
            nc.vector.memset(ones_c[:], 1.0)
            nc.vector.memset(ones_r[:], 1.0)

            ccw_in = dram.tile([2, 1, 2], F32)
            ccw_out = dram.tile([1, 2], F32)
            nc.gpsimd.dma_start(ccw_in[0, :, :], warm[:])
            nc.gpsimd.dma_start(ccw_in[1, :, :], warm[:])
            nc.gpsimd.collective_compute(
                "ReduceScatter", ALU.add,
                ins=[ccw_in.opt()], outs=[ccw_out.opt()],
                replica_groups=[[0, 1], [2, 3], [4, 5], [6, 7]])

            # ================= stage A =================
            with tc.tile_pool(name="stageA", bufs=1) as pa, \
                 tc.tile_pool(name="xhring", bufs=6) as pxh, \
                 tc.tile_pool(name="stag", bufs=4) as pstag, \
                 tc.tile_pool(name="fgring", bufs=3) as pfg, \
                 tc.tile_pool(name="tbring", bufs=2) as ptb:
                xlo_t = pa.tile([128, NP], BF16)
                H0 = pa.tile([128, TLOC, NP], BF16)
                ws_hi_t = pa.tile([128, 128], BF16)
                ws_lo_t = pa.tile([128, 128], BF16)
                w0_hi_t = pa.tile([128, TLOC, 128], BF16)
                w0_lo_t = pa.tile([128, 128], BF16)
                # rank-1 / t'-contraction operands are zero-padded to K=128 so
                # every matmul keeps one PE tile config: mixing (32,128) and
                # (128,128) tiles was measured to halve PE throughput
                nc.vector.memset(ws_lo_t[:], 0.0)
                nc.vector.memset(w0_lo_t[:], 0.0)
                nc.vector.memset(xlo_t[:], 0.0)
                wf_t = pa.tile([128, K, 128], BF16)
                wg_t = pa.tile([128, K, 128], BF16)
                wf8_t = pa.tile([128, 2, 128], F8)
                wg8_t = pa.tile([128, 2, 128], F8)
                H8 = pa.tile([128, TLOC, NP], F8)
                bfv_t = pa.tile([128, 1], F32)
                bgv_t = pa.tile([128, 1], F32)
                nc.sync.dma_start(wf8_t[:], wf8[:])
                nc.sync.dma_start(wg8_t[:], wg8[:])
                nc.sync.dma_start(xlo_t[0:TLOC, :], x_lo[:])
                nc.sync.dma_start(ws_lo_t[0:1, :], wsT_lo[:])
                nc.sync.dma_start(w0_lo_t[0:TLOC, :], w0T_lo[:])
                for t_, d_ in [(ws_hi_t, wsT_hi),
                               (w0_hi_t, w0T_hi),
                               (wf_t, wfT), (wg_t, wgT), (bfv_t, bf_v),
                               (bgv_t, bg_v)]:
                    nc.sync.dma_start(t_[:], d_[:])
                for t_, d_ in [(wmp1_t, wmp1T), (wmp2_t, wmp2T), (w1T_t, w1T),
                               (wET_t, wET), (brv, b_resid_v), (wEs_t, wEsum_v),
                               (b01_t, b01_v), (we1_t, we1T), (be1_t, be1_v),
                               (we2_t, we2T), (be2_t, be2_v), (whT_t, whT),
                               (bh_t, bh_v), (tmask_t, tmask)]:
                    nc.sync.dma_start(t_[:], d_[:])

                # phase 1: start conv (H0) + skip0, looped over t'
                with tc.tile_pool(name="psA1", bufs=5, space="PSUM") as psA1, \
                     tc.tile_pool(name="psA1s", bufs=1, space="PSUM") as psA1s:
                    s0ps = psA1s.tile([128, 3, 512], F32, tag="s0ps")
                    xh_tiles, stg_tiles = [], []
                    for tp_ in range(TLOC):
                        xh = pxh.tile([128, NP], BF16, tag="xh")
                        nc.scalar.dma_start(xh[:], x_hi[:, tp_, :])
                        xh_tiles.append(xh)
                        stg = pstag.tile([128, NP], BF16, tag="xlo_stage")
                        if tp_ < 4:
                            nc.gpsimd.memset(stg[:], 0.0)
                        nc.gpsimd.dma_start(stg[0:1, :], x_lo[tp_:tp_ + 1, :])
                        stg_tiles.append(stg)
                    for tp_ in range(TLOC):
                        xh, stg = xh_tiles[tp_], stg_tiles[tp_]
                        pss = []
                        for ci in range(3):
                            psum = psA1.tile([128, 512], F32, tag="ps_start",
                                             name="ps_start")
                            pss.append(psum)
                        # group matmuls by stationary to keep weight loads off
                        # the critical rate
                        for ci, (vo, vl) in enumerate(VCH):
                            nc.tensor.matmul(pss[ci][:, 0:vl], ws_hi_t[:],
                                             xh[:, vo:vo + vl],
                                             start=True, stop=False)
                        for ci, (vo, vl) in enumerate(VCH):
                            nc.tensor.matmul(pss[ci][:, 0:vl], ws_lo_t[:],
                                             stg[:, vo:vo + vl],
                                             start=False, stop=True)
                        for ci, (vo, vl) in enumerate(VCH):
                            nc.tensor.matmul(s0ps[:, ci, 0:vl], w0_hi_t[:, tp_, :],
                                             xh[:, vo:vo + vl],
                                             start=(tp_ == 0), stop=False)
                        for ci, (vo, vl) in enumerate(VCH):
                            nc.vector.tensor_copy(H0[:, tp_, vo:vo + vl],
                                                  pss[ci][:, 0:vl])
                            nc.scalar.activation(H8[:, tp_, vo:vo + vl],
                                                 pss[ci][:, 0:vl], AF.Copy,
                                                 scale=16.0)
                    for ci, (vo, vl) in enumerate(VCH):
                        # channel-129 skip0 term: contraction over t' in one shot
                        nc.tensor.matmul(s0ps[:, ci, 0:vl], w0_lo_t[:],
                                         xlo_t[:, vo:vo + vl],
                                         start=False, stop=True)
                        nc.vector.tensor_copy(skip_acc[:, vo:vo + vl],
                                              s0ps[:, ci, 0:vl])

                # phase 2: filt/gate inception -> hx2 = 0.5*tanh(.)*sigmoid(.)
                with tc.tile_pool(name="psA2", bufs=6, space="PSUM") as psA2, \
                     tc.tile_pool(name="psA2s", bufs=2, space="PSUM") as psA2s:
                    for tau in range(TAU):
                        psfs, psgs, fss = [], [], []
                        for ci in range(3):
                            psf = psA2.tile([128, 512], F32, tag="ps_fg",
                                            name="ps_fg")
                            psfs.append(psf)
                        for ci, (vo, vl) in enumerate(VCH):
                            nc.tensor.matmul(psfs[ci][:, 0:vl], wf8_t[:],
                                             H8[:, tau:tau + 2, vo:vo + vl],
                                             start=True, stop=False, perf_mode=DR)
                        for ci, (vo, vl) in enumerate(VCH):
                            nc.tensor.matmul(psfs[ci][:, 0:vl], wf_t[:, 2, :],
                                             H0[:, tau + 2, vo:vo + vl],
                                             start=False, stop=True)
                        for ci, (vo, vl) in enumerate(VCH):
                            fs = pfg.tile([128, 512], BF16, tag="fs")
                            nc.scalar.activation(fs[:, 0:vl], psfs[ci][:, 0:vl],
                                                 AF.Tanh, bias=bfv_t[:],
                                                 scale=1.0 / 4096.0)
                            fss.append(fs)
                        for ci in range(3):
                            psg = psA2.tile([128, 512], F32, tag="ps_fg",
                                            name="ps_fg")
                            psgs.append(psg)
                        for ci, (vo, vl) in enumerate(VCH):
                            nc.tensor.matmul(psgs[ci][:, 0:vl], wg8_t[:],
                                             H8[:, tau:tau + 2, vo:vo + vl],
                                             start=True, stop=False, perf_mode=DR)
                        for ci, (vo, vl) in enumerate(VCH):
                            nc.tensor.matmul(psgs[ci][:, 0:vl], wg_t[:, 2, :],
                                             H0[:, tau + 2, vo:vo + vl],
                                             start=False, stop=True)
                        for ci, (vo, vl) in enumerate(VCH):
                            gs = pfg.tile([128, 512], BF16, tag="gs")
                            nc.scalar.activation(gs[:, 0:vl], psgs[ci][:, 0:vl],
                                                 AF.Sigmoid, bias=bgv_t[:],
                                                 scale=1.0 / 4096.0)
                            nc.vector.scalar_tensor_tensor(
                                hx2[:, tau, vo:vo + vl], fss[ci][:, 0:vl], 0.5,
                                gs[:, 0:vl], ALU.mult, ALU.mult)
                        # node-major fp8 copy of x (hop-1 stationary), per tau
                        tb = ptb.tile([128, NV, 128], BF16, tag="xtb")
                        nc.sync.dma_start_transpose(tb[:], hx2[:, tau, :])
                        nc.vector.tensor_scalar_mul(x2T8[:, tau, :, :], tb[:],
                                                    2.0 * SH)

                    # residual (+ start/mixprop biases) into macc, real nodes
                    nc.vector.tensor_scalar_add(macc[:, :, 0:N],
                                                H0[:, 2:TLOC, 0:N], brv[:])
                    if debug_taps:
                        nc.gpsimd.dma_start(taps["d_hx2"][:], hx2[:])
                        nc.gpsimd.dma_start(taps["d_x2T8"][:], x2T8[:])

                    # skip1 conv partial (contract c,tau over local range)
                    for vo, vl in VCH:
                        psum = psA2s.tile([128, 512], F32, tag="ps_s1")
                        for tau in range(TAU):
                            nc.tensor.matmul(psum[:, 0:vl], w1T_t[:, tau, :],
                                             hx2[0:CC, tau, vo:vo + vl],
                                             start=(tau == 0), stop=(tau == TAU - 1))
                        nc.vector.tensor_tensor(skip_acc[:, vo:vo + vl],
                                                skip_acc[:, vo:vo + vl],
                                                psum[:, 0:vl], op=ALU.add)

            cc_in = dram.tile([2, 128, 2 * NH + 2], BF16)
            cc_out = dram.tile([128, 2 * NH + 2], BF16)

            # ================= mixprop =================
            # per-tau chains h_{k+1} = G h_k + 0.5x, software-pipelined (skew 3):
            # step t issues hop1(t), hop2(t-1), hop3(t-2), conv1x1+stats(t-3)
            with tc.tile_pool(name="mx", bufs=4) as mx, \
                 tc.tile_pool(name="mxg", bufs=1) as mxg, \
                 tc.tile_pool(name="mxb", bufs=3) as mxb, \
                 tc.tile_pool(name="mx8", bufs=4) as mx8, \
                 tc.tile_pool(name="mscr", bufs=2) as mscr, \
                 tc.tile_pool(name="psH", bufs=4, space="PSUM") as psH, \
                 tc.tile_pool(name="psC", bufs=1, space="PSUM") as psC, \
                 tc.tile_pool(name="psR", bufs=1, space="PSUM") as psR:

                g8a_t = mxg.tile([128, NV2, 2, NP], F8, tag="g8a", name="g8a_t")
                g8b_t = mxg.tile([128, NV2, 2, NP], F8, tag="g8b", name="g8b_t")
                nc.gpsimd.dma_start(g8a_t[:], g8_1[:])
                nc.gpsimd.dma_start(g8b_t[:], g8_2[:])
                # stage skip01 partials into the (single, tail) collective
                # payload: [0:NH]=rawE half, [NH:2NH]=skip half, then stats
                nc.gpsimd.dma_start(cc_in[0, :, NH:2 * NH], skip_acc[:, 0:NH])
                nc.gpsimd.dma_start(cc_in[1, :, NH:2 * NH], skip_acc[:, NH:NP])

                def emit_hop(g8, stat, hc, kk, tau, mp):
                    # hc[:, kk, :] = INV * (stat.T-pairs @ g8) + hx2[:, tau, :]
                    for vo, vl in VCH:
                        ps = psH.tile([128, 512], F32, tag="ps_hop")
                        for m in range(NV2):
                            nc.tensor.matmul(ps[:, 0:vl],
                                             stat[:, 2 * m:2 * m + 2, :],
                                             g8[:, m, :, vo:vo + vl],
                                             start=(m == 0), stop=(m == NV2 - 1),
                                             perf_mode=DR)
                        nc.vector.scalar_tensor_tensor(
                            hc[:, kk, vo:vo + vl], ps[:, 0:vl], INV,
                            hx2[:, tau, vo:vo + vl], ALU.mult, ALU.add)
                    if debug_taps and mp == 0 and tau == 0:
                        nc.gpsimd.dma_start(taps[f"d_h{kk + 1}"][:], hc[:, kk, :])
                    if kk == 2:
                        return None
                    # node-major fp8 copy for the next hop's stationary
                    tb = mxb.tile([128, NV, 128], BF16, tag="htb")
                    nc.sync.dma_start_transpose(tb[:], hc[:, kk, :])
                    t8 = mx8.tile([128, NV, 128], F8, tag="ht8")
                    nc.scalar.activation(t8[:], tb[:], AF.Copy, scale=SH)
                    return t8

                for mp in range(2):
                    g8 = g8a_t if mp == 0 else g8b_t
                    wmp = wmp1_t if mp == 0 else wmp2_t
                    st = {}
                    rEps = {}
                    for step in range(TAU + 3):
                        t1 = step
                        if t1 < TAU:
                            hc = mx.tile([128, 3, NP], BF16, tag="hcm3")
                            st[t1] = [hc, None, None]
                            st[t1][1] = emit_hop(g8, x2T8[:, t1, :, :], hc, 0,
                                                 t1, mp)
                        t2 = step - 1
                        if 0 <= t2 < TAU:
                            hc = st[t2][0]
                            st[t2][2] = emit_hop(g8, st[t2][1][:], hc, 1, t2, mp)
                        t3 = step - 2
                        if 0 <= t3 < TAU:
                            emit_hop(g8, st[t3][2][:], st[t3][0], 2, t3, mp)
                        t4 = step - 3
                        if 0 <= t4 < TAU:
                            hc = st.pop(t4)[0]
                            for vo, vl in VCH:
                                ps = psC.tile([128, 512], F32, tag="ps_cv")
                                if mp == 0:
                                    nc.tensor.matmul(ps[:, 0:vl], wmp[:, 0, :],
                                                     hx2[:, t4, vo:vo + vl],
                                                     start=True, stop=False)
                                for k in range(3):
                                    nc.tensor.matmul(ps[:, 0:vl], wmp[:, k + 1, :],
                                                     hc[:, k, vo:vo + vl],
                                                     start=(mp == 1 and k == 0),
                                                     stop=(k == 2))
                                hi = min(vo + vl, N)
                                nc.vector.tensor_tensor(
                                    macc[:, t4, vo:hi], macc[:, t4, vo:hi],
                                    ps[:, 0:hi - vo], op=ALU.add)
                            if mp == 1:
                                # macc row t4 final: stats + rawE contribution
                                nc.vector.reduce_sum(sums_t[:, t4:t4 + 1],
                                                     macc[:, t4, :],
                                                     axis=mybir.AxisListType.X)
                                scr = mscr.tile([128, NP], BF16, tag="sq_scr")
                                nc.scalar.activation(scr[:], macc[:, t4, :],
                                                     AF.Square,
                                                     scale=tmask_t[:, t4:t4 + 1],
                                                     accum_out=sqs_t[:, t4:t4 + 1])
                                for ci, (vo, vl) in enumerate(VCH):
                                    if t4 == 0:
                                        rEps[ci] = psR.tile(
                                            [128, 512], F32,
                                            tag=f"ps_rE{ci}", name=f"ps_rE{ci}")
                                    nc.tensor.matmul(rEps[ci][:, 0:vl],
                                                     wET_t[:, t4, :],
                                                     macc[:, t4, vo:vo + vl],
                                                     start=(t4 == 0),
                                                     stop=(t4 == TAU - 1))
                                    if t4 == TAU - 1:
                                        nc.vector.tensor_copy(
                                            rawE_acc[:, vo:vo + vl],
                                            rEps[ci][:, 0:vl])

            if debug_taps:
                nc.gpsimd.dma_start(taps["d_macc"][:], macc[:])
                nc.gpsimd.dma_start(taps["d_skip"][:], skip_acc[:])
                nc.gpsimd.dma_start(taps["d_rawE"][:], rawE_acc[:])
            # re-warm act tables for the tail under the collective's shadow
            nc.scalar.activation(warm_o[:], warm[:], AF.Sqrt)
            # ================= stats + collective + end convs =================
            with tc.tile_pool(name="late", bufs=1) as pl, \
                 tc.tile_pool(name="psL", bufs=1, space="PSUM") as ps:
                stats_p = pl.tile([128, 2], F32)
                msum = pl.tile([128, TAU], F32)
                nc.vector.tensor_tensor(msum[:], sums_t[:], tmask_t[:], op=ALU.mult)
                nc.vector.reduce_sum(stats_p[:, 0:1], msum[:],
                                     axis=mybir.AxisListType.X)
                nc.vector.reduce_sum(stats_p[:, 1:2], sqs_t[:],
                                     axis=mybir.AxisListType.X)
                if debug_taps:
                    nc.gpsimd.dma_start(taps["d_stats"][:], stats_p[:])

                # cross-partition reduce + broadcast of stats on the PE
                # (avoids a gpsimd custom-op library reload on the tail)
                ps_s0 = ps.tile([1, 2], F32, tag="ps_s0")
                nc.tensor.matmul(ps_s0[:], ones_c[:], stats_p[:],
                                 start=True, stop=True)
                st01 = pl.tile([1, 2], F32)
                nc.vector.tensor_copy(st01[:], ps_s0[:])
                ps_s1 = ps.tile([128, 2], F32, tag="ps_s1")
                nc.tensor.matmul(ps_s1[:], ones_r[:], st01[:],
                                 start=True, stop=True)
                st_loc = pl.tile([128, 2], BF16)
                nc.vector.tensor_copy(st_loc[:], ps_s1[:])

                nc.gpsimd.dma_start(cc_in[0, :, 0:NH], rawE_acc[:, 0:NH])
                nc.gpsimd.dma_start(cc_in[1, :, 0:NH], rawE_acc[:, NH:NP])
                nc.gpsimd.dma_start(cc_in[0, :, 2 * NH:2 * NH + 2], st_loc[:])
                nc.gpsimd.dma_start(cc_in[1, :, 2 * NH:2 * NH + 2], st_loc[:])
                nc.gpsimd.collective_compute(
                    "ReduceScatter", ALU.add,
                    ins=[cc_in.opt()], outs=[cc_out.opt()],
                    replica_groups=[[0, 1], [2, 3], [4, 5], [6, 7]])
                rawE_c = pl.tile([128, 2 * NH + 2], BF16)
                nc.gpsimd.dma_start(rawE_c[:, 2 * NH:2 * NH + 2],
                                    cc_out[:, 2 * NH:2 * NH + 2])
                nc.scalar.dma_start(rawE_c[:, 0:2 * NH], cc_out[:, 0:2 * NH])

                # layernorm scalars (same value on every partition)
                mv = pl.tile([128, 1], F32)
                msqv = pl.tile([128, 1], F32)
                varv = pl.tile([128, 1], F32)
                m2v = pl.tile([128, 1], F32)
                svv = pl.tile([128, 1], F32)
                rv = pl.tile([128, 1], F32)
                rmv = pl.tile([128, 1], F32)
                bias_c = pl.tile([128, 1], F32)
                nc.vector.tensor_scalar_mul(mv[:], rawE_c[:, 2 * NH:2 * NH + 1],
                                            1.0 / CNT)
                nc.vector.tensor_scalar_mul(msqv[:],
                                            rawE_c[:, 2 * NH + 1:2 * NH + 2],
                                            1.0 / CNT)
                nc.vector.tensor_tensor(m2v[:], mv[:], mv[:], op=ALU.mult)
                nc.vector.tensor_scalar(varv[:], msqv[:], m2v[:], EPS,
                                        op0=ALU.subtract, op1=ALU.add)
                nc.scalar.sqrt(svv[:], varv[:])
                nc.vector.reciprocal(rv[:], svv[:])
                nc.vector.tensor_scalar(rmv[:], rv[:], mv[:], -1.0,
                                        op0=ALU.mult, op1=ALU.mult)
                # bias_c = b01 - r*m*wEsum
                nc.vector.scalar_tensor_tensor(bias_c[:], wEs_t[:], rmv[:],
                                               b01_t[:], ALU.mult, ALU.add)
                # skip_pre = skip01 + r*rawE ; relu with bias; end convs +
                # head, pipelined per node chunk so the serial chain overlaps
                skip_pre = pl.tile([128, NH], F32)
                rsk = pl.tile([128, NH], BF16)
                o1 = pl.tile([128, NH], BF16)
                o2 = pl.tile([OUT, NH], BF16)
                y_sb = pl.tile([1, NH], F32)
                ps1 = ps.tile([128, NH], F32, tag="ps_e1")
                ps2 = ps.tile([OUT, NH], F32, tag="ps_e2")
                psh = ps.tile([1, NH], F32, tag="ps_eh")
                for vo, vl in ECH:
                    nc.vector.scalar_tensor_tensor(
                        skip_pre[:, vo:vo + vl], rawE_c[:, vo:vo + vl],
                        rv[:], rawE_c[:, NH + vo:NH + vo + vl],
                        ALU.mult, ALU.add)
                    nc.vector.tensor_scalar(rsk[:, vo:vo + vl],
                                            skip_pre[:, vo:vo + vl],
                                            bias_c[:], 0.0,
                                            op0=ALU.add, op1=ALU.max)
                for vo, vl in ECH:
                    nc.tensor.matmul(ps1[:, vo:vo + vl], we1_t[:],
                                     rsk[:, vo:vo + vl], start=True, stop=True)
                    nc.vector.tensor_scalar(o1[:, vo:vo + vl],
                                            ps1[:, vo:vo + vl], be1_t[:], 0.0,
                                            op0=ALU.add, op1=ALU.max)
                for vo, vl in ECH:
                    nc.tensor.matmul(ps2[:, vo:vo + vl], we2_t[:],
                                     o1[:, vo:vo + vl], start=True, stop=True)
                    nc.vector.tensor_scalar_add(o2[:, vo:vo + vl],
                                                ps2[:, vo:vo + vl], be2_t[:])
                for vo, vl in ECH:
                    nc.tensor.matmul(psh[:, vo:vo + vl], whT_t[:],
                                     o2[:, vo:vo + vl], start=True, stop=True)
                nc.scalar.activation(y_sb[:], psh[:], AF.Sigmoid,
                                     bias=bh_t[:], scale=1.0)
                nc.gpsimd.dma_start(y[:], y_sb[:])

    nc.compile()
    return nc


def _wk2(w):
    # k-taps as [c, k, o(pad 128)]; tap 2 carries the 4096x psum scale used by
    # the fp8 DoubleRow path for taps 0/1
    arr = np.pad(w.transpose(1, 2, 0), ((0, 0), (0, 0), (0, 2)))
    arr[:, 2, :] *= 4096.0
    return arr.astype(bf16)


def _w8(w):
    arr = np.pad(w.transpose(1, 2, 0)[:, 0:2, :], ((0, 0), (0, 0), (0, 2)))
    return np.clip(arr * 256.0, -240.0, 240.0).astype(f8e4)


def _norm_adj_T_g8(a):
    """SG * 0.5 * norm_adj(a).T zero-padded, packed for fp8 DoubleRow rhs."""
    an = a + np.eye(N, dtype=np.float32)
    an = an / an.sum(axis=1, keepdims=True)
    g = (0.5 * SG) * an.T
    gp = np.zeros((NP, NP), dtype=np.float32)
    gp[:N, :N] = g
    np.clip(gp, -240.0, 240.0, out=gp)
    # w = 256*m + 128*j + p  ->  [p, m, j, v]
    return gp.reshape(NV2, 2, 128, NP).transpose(2, 0, 1, 3).astype(f8e4)


def _prep_inputs(inputs):
    x = np.asarray(inputs["x"], np.float32)
    adj = np.asarray(inputs["adj"], np.float32)
    w_start = np.asarray(inputs["w_start"], np.float32)
    b_start = np.asarray(inputs["b_start"], np.float32)
    w_filt = np.asarray(inputs["w_filt"], np.float32)[:, :, 0, :]
    b_filt = np.asarray(inputs["b_filt"], np.float32)
    w_gate = np.asarray(inputs["w_gate"], np.float32)[:, :, 0, :]
    b_gate = np.asarray(inputs["b_gate"], np.float32)
    w_skip0 = np.asarray(inputs["w_skip0"], np.float32)[:, :, 0, :]
    b_skip0 = np.asarray(inputs["b_skip0"], np.float32)
    w_skip1 = np.asarray(inputs["w_skip1"], np.float32)[:, :, 0, :]
    b_skip1 = np.asarray(inputs["b_skip1"], np.float32)
    w_mp1 = np.asarray(inputs["w_mp1"], np.float32)
    b_mp1 = np.asarray(inputs["b_mp1"], np.float32)
    w_mp2 = np.asarray(inputs["w_mp2"], np.float32)
    b_mp2 = np.asarray(inputs["b_mp2"], np.float32)
    w_skipE = np.asarray(inputs["w_skipE"], np.float32)[:, :, 0, :]
    b_skipE = np.asarray(inputs["b_skipE"], np.float32)
    w_end1 = np.asarray(inputs["w_end1"], np.float32)
    b_end1 = np.asarray(inputs["b_end1"], np.float32)
    w_end2 = np.asarray(inputs["w_end2"], np.float32)
    b_end2 = np.asarray(inputs["b_end2"], np.float32)
    w_head = np.asarray(inputs["w_head"], np.float32)
    b_head = np.asarray(inputs["b_head"], np.float32)

    g8_1 = _norm_adj_T_g8(adj)
    g8_2 = _norm_adj_T_g8(adj.T)

    # shared (core-independent) tensors
    wsT = w_start.T  # [129, 128]
    shared = {
        "g8_1": g8_1, "g8_2": g8_2,
        "wsT_hi": wsT[:128].astype(bf16),
        "wsT_lo": wsT[128:129].astype(bf16),
        "wfT": _wk2(w_filt), "wgT": _wk2(w_gate),
        "wf8": _w8(w_filt), "wg8": _w8(w_gate),
        "bf_v": np.pad((b_filt + w_filt.sum(2) @ b_start), (0, 2)).reshape(128, 1).astype(np.float32),
        "bg_v": np.pad((b_gate + w_gate.sum(2) @ b_start), (0, 2)).reshape(128, 1).astype(np.float32),
        "b_resid_v": (b_start + b_mp1 + b_mp2).reshape(128, 1).astype(np.float32),
        "wEsum_v": w_skipE.sum((1, 2)).reshape(128, 1).astype(np.float32),
        "b01_v": (b_skip0 + b_skip1 + b_skipE).reshape(128, 1).astype(np.float32),
        "we1T": w_end1.T.astype(bf16),
        "be1_v": b_end1.reshape(128, 1).astype(np.float32),
        "we2T": w_end2.T.astype(bf16),
        "be2_v": b_end2.reshape(OUT, 1).astype(np.float32),
        "whT": w_head.T.astype(bf16),
        "bh_v": b_head.reshape(1, 1).astype(np.float32),
    }
    # w_mp as [c(128 pad), k, o]; k=0 slot 2x (it multiplies hx2 = x/2) and
    # carries BOTH mixprops' x-terms (applied once, in the mp=0 conv)
    for nm, w, wo in (("wmp1T", w_mp1, w_mp2), ("wmp2T", w_mp2, None)):
        arr = np.zeros((128, 4, 128), np.float32)
        for k in range(4):
            arr[:CC, k, :] = w[:, k * CC:(k + 1) * CC].T
        arr[:, 0, :] *= 2.0
        if wo is not None:
            arr[:CC, 0, :] += 2.0 * wo[:, 0:CC].T
        shared[nm] = arr.astype(bf16)

    in_maps = []
    for core in range(8):
        b, th = core // 2, core % 2
        t_lo = 0 if th == 0 else TAU
        # x slice [129, 1280, TLOC] zero-padded in nodes and t
        xp = np.zeros((C_IN, TLOC, NP), np.float32)
        t_hi = min(t_lo + TLOC, T)
        xp[:, 0:t_hi - t_lo, :N] = x[b, :, :, t_lo:t_hi].transpose(0, 2, 1)
        # skip0 weight slots aligned to local t: core owns t range
        w0T = np.zeros((C_IN, TLOC, 128), np.float32)
        own_lo, own_hi = (0, 13) if th == 0 else (13, T)
        for tp_ in range(TLOC):
            tg = t_lo + tp_
            if own_lo <= tg < own_hi:
                w0T[:, tp_, :] = w_skip0[:, :, tg].T
        # skip1 / skipE weight slots aligned to local tau
        w1Ta = np.zeros((CC, TAU, 128), np.float32)
        wETa = np.zeros((128, TAU, 128), np.float32)
        for tau in range(TAU):
            tg = t_lo + tau
            if tg < T1:
                w1Ta[:, tau, :] = 2.0 * w_skip1[:, :, tg].T  # reads hx2 = x/2
                wETa[:, tau, :] = w_skipE[:, :, tg].T
        tm = np.ones((128, TAU), np.float32)
        if th == 1:
            tm[:, T1 - TAU:] = 0.0  # tau slots beyond T1 are padding
        m = dict(shared)
        m["x_hi"] = xp[:128].astype(bf16)
        m["x_lo"] = xp[128].astype(bf16)
        m["w0T_hi"] = w0T[:128].astype(bf16)
        m["w0T_lo"] = w0T[128].astype(bf16)
        m["w1T"] = w1Ta.astype(bf16)
        m["wET"] = wETa.astype(bf16)
        m["tmask"] = tm
        in_maps.append(m)
    return in_maps


def kernel(**inputs):
    if "nc" not in _CACHE:
        _CACHE["nc"] = _build_program()
    nc = _CACHE["nc"]
    in_maps = _prep_inputs(inputs)
    res = bass_utils.run_bass_kernel_spmd(nc, in_maps, core_ids=list(range(8)))
    out = np.empty((B, N), np.float32)
    for b in range(B):
        out[b, 0:NH] = res.results[2 * b]["y"][0]
        out[b, NH:N] = res.results[2 * b + 1]["y"][0, 0:N - NH]
    return out
